# revision 2
# baseline (speedup 1.0000x reference)
"""Trainium2 Bass kernel for the DiffusionProcess problem.

Strategy (hardcoded for B=2048, R=512, Z=256, H=512, T=16, 8 cores):
  - Data parallel: batch sharded 8 x 256, MLP weights replicated.
  - Feature-major layout on device: activations stored [feature, batch]
    so matmuls are out[M,N] = W[K,M].T @ x[K,N] with K,M tiles of 128 and
    N = 256 (the per-core batch) and biases are per-partition columns.
  - Matmuls run in float32r (TF32) at 1 PE cycle/row.
  - r @ W0[Z:] is step-invariant -> computed once before the scan.
  - temb_t @ W0 + b0 is batch-invariant -> precomputed for all 16 steps
    as [H, 16] columns, used as per-partition bias.
  - Per step: 4 chained matmul stages (k-outer so accumulation of stage
    i+1 starts as soon as the first activation k-tile of stage i is
    ready), fused elementwise via scalar_tensor_tensor on DVE and
    Identity+bias on ACT.
"""

import sys

if "/opt/trn_rl_repo" not in sys.path:
    sys.path.insert(0, "/opt/trn_rl_repo")

import numpy as np

B, R, Z, H = 2048, 512, 256, 512
ZR = Z + R
T = 16
NC = 8
BS = B // NC          # 256 batch per core
DT = 1.0 / T
SQDT = DT ** 0.5
P = 128
KZ = Z // P           # 2
KR = R // P           # 4
KH = H // P           # 4
MH = H // P           # 4
MZ = Z // P           # 2

_CACHE = {}


def _build():
    import concourse.bacc as bacc
    import concourse.tile as tile
    from concourse import mybir

    F32 = mybir.dt.float32
    F32R = mybir.dt.float32r
    AF = mybir.ActivationFunctionType
    OP = mybir.AluOpType

    nc = bacc.Bacc("TRN2", target_bir_lowering=False, debug=False,
                   num_devices=NC)

    # ---- DRAM tensors (per-core views; weights replicated) ----
    d_wz = nc.dram_tensor("wz", [Z, H], F32R, kind="ExternalInput").ap()
    d_wr = nc.dram_tensor("wr", [R, H], F32R, kind="ExternalInput").ap()
    d_wh = nc.dram_tensor("wh", [H, H], F32R, kind="ExternalInput").ap()
    d_wo = nc.dram_tensor("wo", [H, Z], F32R, kind="ExternalInput").ap()
    d_wt = nc.dram_tensor("wt", [1, ZR], F32R, kind="ExternalInput").ap()
    d_ts = nc.dram_tensor("ts", [1, T], F32R, kind="ExternalInput").ap()
    d_ones = nc.dram_tensor("ones", [1, BS], F32R, kind="ExternalInput").ap()
    d_bo = nc.dram_tensor("bo_row", [1, Z], F32R, kind="ExternalInput").ap()
    d_bt = nc.dram_tensor("bt", [ZR, 1], F32, kind="ExternalInput").ap()
    d_b0 = nc.dram_tensor("b0", [H, 1], F32, kind="ExternalInput").ap()
    d_bh = nc.dram_tensor("bh", [H, 1], F32, kind="ExternalInput").ap()
    d_rt = nc.dram_tensor("rT", [R, BS], F32R, kind="ExternalInput").ap()
    d_z0 = nc.dram_tensor("z0T", [Z, BS], F32R, kind="ExternalInput").ap()
    d_eps = nc.dram_tensor("epsT", [T, Z, BS], F32, kind="ExternalInput").ap()
    d_zs = nc.dram_tensor("zsT", [T, Z, BS], F32R, kind="ExternalOutput").ap()
    d_mus = nc.dram_tensor("musT", [T, Z, BS], F32, kind="ExternalOutput").ap()

    with tile.TileContext(nc) as tc:
        with tc.tile_pool(name="w", bufs=1) as wp, \
             tc.tile_pool(name="v", bufs=1) as vp, \
             tc.tile_pool(name="act", bufs=1) as ap_, \
             tc.tile_pool(name="st", bufs=2) as sp, \
             tc.tile_pool(name="ps", bufs=1, space="PSUM") as pp:

            # ---- weight / constant loads ----
            wz = [wp.tile([P, H], F32R, tag=f"wz{k}", name=f"wz{k}") for k in range(KZ)]
            for k in range(KZ):
                nc.sync.dma_start(wz[k][:], d_wz[k * P:(k + 1) * P, :])
            wr = [wp.tile([P, H], F32R, tag=f"wr{k}", name=f"wr{k}") for k in range(KR)]
            for k in range(KR):
                nc.sync.dma_start(wr[k][:], d_wr[k * P:(k + 1) * P, :])
            wh = [wp.tile([P, H], F32R, tag=f"wh{k}", name=f"wh{k}") for k in range(KH)]
            for k in range(KH):
                nc.sync.dma_start(wh[k][:], d_wh[k * P:(k + 1) * P, :])
            wo = [wp.tile([P, Z], F32R, tag=f"wo{k}", name=f"wo{k}") for k in range(KH)]
            for k in range(KH):
                nc.sync.dma_start(wo[k][:], d_wo[k * P:(k + 1) * P, :])

            wt = vp.tile([1, ZR], F32R, tag="wt", name="wt")
            nc.sync.dma_start(wt[:], d_wt[:])
            ts = vp.tile([1, T], F32R, tag="ts", name="ts")
            nc.sync.dma_start(ts[:], d_ts[:])
            ones = vp.tile([1, BS], F32R, tag="ones", name="ones")
            nc.sync.dma_start(ones[:], d_ones[:])
            bo = vp.tile([1, Z], F32R, tag="bo", name="bo")
            nc.sync.dma_start(bo[:], d_bo[:])
            bt = [vp.tile([P, 1], F32, tag=f"bt{f}", name=f"bt{f}") for f in range(ZR // P)]
            for f in range(ZR // P):
                nc.sync.dma_start(bt[f][:], d_bt[f * P:(f + 1) * P, :])
            b0 = [vp.tile([P, 1], F32, tag=f"b0{m}", name=f"b0{m}") for m in range(MH)]
            for m in range(MH):
                nc.sync.dma_start(b0[m][:], d_b0[m * P:(m + 1) * P, :])
            bh = [vp.tile([P, 1], F32, tag=f"bh{m}", name=f"bh{m}") for m in range(MH)]
            for m in range(MH):
                nc.sync.dma_start(bh[m][:], d_bh[m * P:(m + 1) * P, :])

            rt = [wp.tile([P, BS], F32R, tag=f"rt{k}", name=f"rt{k}") for k in range(KR)]
            for k in range(KR):
                nc.sync.dma_start(rt[k][:], d_rt[k * P:(k + 1) * P, :])

            # initial z state
            z = [sp.tile([P, BS], F32R, tag=f"z{k}", name=f"z{k}") for k in range(KZ)]
            for k in range(KZ):
                nc.sync.dma_start(z[k][:], d_z0[k * P:(k + 1) * P, :])

            # W0 row-tiles in feature order (z features then r features)
            w0 = wz + wr  # 6 tiles of [128, H]

            # ---- temb[f] = relu(Wt_f^T ts + bt_f) : [128, T] ----
            temb = [ap_.tile([P, T], F32R, tag=f"temb{f}", name=f"temb{f}")
                    for f in range(ZR // P)]
            for f in range(ZR // P):
                ps = pp.tile([P, T], F32, tag=f"pa{f % MH}", name=f"pa{f % MH}")
                nc.tensor.matmul(ps[:], wt[0:1, f * P:(f + 1) * P], ts[:],
                                 start=True, stop=True)
                nc.scalar.activation(temb[f][:], ps[:], AF.Relu, bias=bt[f][:])

            # ---- c[m][:, t] = (temb_t @ W0 + b0)[m-tile] : [128, T] ----
            c = [ap_.tile([P, T], F32, tag=f"c{m}", name=f"c{m}") for m in range(MH)]
            for m in range(MH):
                ps = pp.tile([P, T], F32, tag=f"pb{m}", name=f"pb{m}")
                for f in range(ZR // P):
                    nc.tensor.matmul(ps[:], w0[f][:, m * P:(m + 1) * P],
                                     temb[f][:], start=(f == 0),
                                     stop=(f == ZR // P - 1))
                nc.scalar.activation(c[m][:], ps[:], AF.Identity,
                                     bias=b0[m][:])

            # ---- rW[m] = (r @ W0[Z:])[m-tile].T : [128, BS] ----
            rw = [ap_.tile([P, BS], F32, tag=f"rw{m}", name=f"rw{m}") for m in range(MH)]
            for m in range(MH):
                ps = pp.tile([P, BS], F32, tag=f"pa{m}", name=f"pa{m}")
                for k in range(KR):
                    nc.tensor.matmul(ps[:], wr[k][:, m * P:(m + 1) * P],
                                     rt[k][:], start=(k == 0),
                                     stop=(k == KR - 1))
                nc.scalar.activation(rw[m][:], ps[:], AF.Copy)

            # ---- the scan ----
            for t in range(T):
                eps = [sp.tile([P, BS], F32, tag=f"e{k}", name=f"e{k}", bufs=3)
                       for k in range(KZ)]
                for k in range(KZ):
                    nc.sync.dma_start(eps[k][:],
                                      d_eps[t, k * P:(k + 1) * P, :])

                # stage A: ps_a[m] = z @ Wz tiles; a = relu(ps_a + c_t + rW)
                ps_a = [pp.tile([P, BS], F32, tag=f"pa{m}", name=f"pa{m}")
                        for m in range(MH)]
                for k in range(KZ):
                    for m in range(MH):
                        nc.tensor.matmul(ps_a[m][:],
                                         wz[k][:, m * P:(m + 1) * P],
                                         z[k][:], start=(k == 0),
                                         stop=(k == KZ - 1))
                a = []
                for m in range(MH):
                    tmp = sp.tile([P, BS], F32, tag=f"tmp{m}", name=f"tmp{m}", bufs=1)
                    nc.vector.scalar_tensor_tensor(
                        tmp[:], ps_a[m][:], c[m][:, t:t + 1], rw[m][:],
                        op0=OP.add, op1=OP.add)
                    at = sp.tile([P, BS], F32R, tag=f"a{m}", name=f"a{m}", bufs=1)
                    nc.vector.tensor_scalar_max(at[:], tmp[:], 0.0)
                    a.append(at)

                # stage B: g1 = a @ Wh + bh
                ps_b = [pp.tile([P, BS], F32, tag=f"pb{m}", name=f"pb{m}")
                        for m in range(MH)]
                for k in range(KH):
                    for m in range(MH):
                        nc.tensor.matmul(ps_b[m][:],
                                         wh[k][:, m * P:(m + 1) * P],
                                         a[k][:], start=(k == 0),
                                         stop=(k == KH - 1))
                g1 = []
                for m in range(MH):
                    g = sp.tile([P, BS], F32R, tag=f"g1{m}", name=f"g1{m}", bufs=1)
                    nc.scalar.activation(g[:], ps_b[m][:], AF.Identity,
                                         bias=bh[m][:])
                    g1.append(g)

                # stage C: g2 = g1 @ Wh + bh
                ps_c = [pp.tile([P, BS], F32, tag=f"pa{m}", name=f"pa{m}")
                        for m in range(MH)]
                for k in range(KH):
                    for m in range(MH):
                        nc.tensor.matmul(ps_c[m][:],
                                         wh[k][:, m * P:(m + 1) * P],
                                         g1[k][:], start=(k == 0),
                                         stop=(k == KH - 1))
                g2 = []
                for m in range(MH):
                    g = sp.tile([P, BS], F32R, tag=f"g2{m}", name=f"g2{m}", bufs=1)
                    nc.scalar.activation(g[:], ps_c[m][:], AF.Identity,
                                         bias=bh[m][:])
                    g2.append(g)

                # stage D: s = g2 @ Wo + bo (bo via rank-1 ones matmul)
                ps_d = [pp.tile([P, BS], F32, tag=f"pb{m}", name=f"pb{m}")
                        for m in range(MZ)]
                for k in range(KH):
                    for m in range(MZ):
                        nc.tensor.matmul(ps_d[m][:],
                                         wo[k][:, m * P:(m + 1) * P],
                                         g2[k][:], start=(k == 0),
                                         stop=False)
                for m in range(MZ):
                    nc.tensor.matmul(ps_d[m][:], bo[0:1, m * P:(m + 1) * P],
                                     ones[:], start=False, stop=True)

                # mu = dt * s + z ; z' = sqdt * eps + mu
                z_new = []
                for m in range(MZ):
                    mu = sp.tile([P, BS], F32, tag=f"mu{m}", name=f"mu{m}", bufs=2)
                    nc.vector.scalar_tensor_tensor(
                        mu[:], ps_d[m][:], DT, z[m][:],
                        op0=OP.mult, op1=OP.add)
                    zn = sp.tile([P, BS], F32R, tag=f"z{m}", name=f"z{m}")
                    nc.vector.scalar_tensor_tensor(
                        zn[:], eps[m][:], SQDT, mu[:],
                        op0=OP.mult, op1=OP.add)
                    z_new.append(zn)
                    nc.sync.dma_start(d_mus[t, m * P:(m + 1) * P, :], mu[:])
                    nc.sync.dma_start(d_zs[t, m * P:(m + 1) * P, :], zn[:])
                z = z_new

    nc.compile()
    return nc


def _get_nc():
    if "nc" not in _CACHE:
        _CACHE["nc"] = _build()
    return _CACHE["nc"]


def _in_maps(inputs):
    f32 = lambda x: np.ascontiguousarray(np.asarray(x, dtype=np.float32))
    r = f32(inputs["r"])
    noise0 = f32(inputs["noise0"])
    noise = f32(inputs["noise"])
    W0 = f32(inputs["W0"])
    b0 = f32(inputs["b0"])
    Wh = f32(inputs["Wh"])
    bh = f32(inputs["bh"])
    Wo = f32(inputs["Wo"])
    bo = f32(inputs["bo"])
    Wt = f32(inputs["Wt"])
    bt = f32(inputs["bt"])

    shared = {
        "wz": W0[:Z], "wr": W0[Z:], "wh": Wh, "wo": Wo,
        "wt": Wt.reshape(1, ZR),
        "ts": (np.arange(1, T + 1, dtype=np.float32) * DT).reshape(1, T),
        "ones": np.ones((1, BS), np.float32),
        "bo_row": bo.reshape(1, Z),
        "bt": bt.reshape(ZR, 1),
        "b0": b0.reshape(H, 1),
        "bh": bh.reshape(H, 1),
    }
    rT = np.ascontiguousarray(r.T)                    # [R, B]
    z0T = np.ascontiguousarray(noise0.T)              # [Z, B]
    epsT = np.ascontiguousarray(noise.transpose(0, 2, 1))  # [T, Z, B]
    maps = []
    for cix in range(NC):
        s = slice(cix * BS, (cix + 1) * BS)
        m = dict(shared)
        m["rT"] = np.ascontiguousarray(rT[:, s])
        m["z0T"] = np.ascontiguousarray(z0T[:, s])
        m["epsT"] = np.ascontiguousarray(epsT[:, :, s])
        maps.append(m)
    return maps, noise0


def _run(inputs, **run_kwargs):
    from concourse.bass_utils import run_bass_kernel_spmd
    nc = _get_nc()
    maps, noise0 = _in_maps(inputs)
    res = run_bass_kernel_spmd(nc, maps, core_ids=list(range(NC)),
                               **run_kwargs)
    out = np.empty((3, T + 1, B, Z), np.float32)
    out[0, 0] = noise0
    out[1, 0] = 0.0
    out[2, 0] = 1.0
    out[2, 1:] = SQDT
    for cix in range(NC):
        s = slice(cix * BS, (cix + 1) * BS)
        out[0, 1:, s, :] = res.results[cix]["zsT"].transpose(0, 2, 1)
        out[1, 1:, s, :] = res.results[cix]["musT"].transpose(0, 2, 1)
    return out, res


def kernel(**inputs) -> np.ndarray:
    out, _ = _run(inputs)
    return out


# revision 4
# speedup vs baseline: 1.2202x; 1.2202x over previous
"""Trainium2 Bass kernel for the DiffusionProcess problem.

Strategy (hardcoded for B=2048, R=512, Z=256, H=512, T=16, 8 cores):
  - Data parallel: batch sharded 8 x 256, MLP weights replicated.
  - Feature-major layout on device: activations stored [feature, batch]
    so matmuls are out[M,N] = W[K,M].T @ x[K,N] with K,M tiles of 128 and
    N = 256 (the per-core batch) and biases are per-partition columns.
  - Matmuls run in float32r (TF32) at ~1.3 PE cycles/row.
  - r @ W0[Z:] is step-invariant -> computed once before the scan.
  - temb_t @ W0 + b0 is batch-invariant -> precomputed for all 16 steps
    as [H, 16] columns, used as per-partition bias.
  - Step-boundary retiming: y = z + sqrt_dt*eps + dt*bo is precomputed
    off the critical path (eps is an input, known ahead), so between the
    last Wo matmul of step t and the first Wz matmul of step t+1 there
    is only ONE fused DVE op: z' = dt*psum + y. The reference's mu is
    reconstructed off-path as mu = z' - sqrt_dt*eps.
  - k-outer accumulation order so stage i+1's matmuls can start as soon
    as the first activation k-tile of stage i is ready (keeps the PE
    dense -> HAM stays at full clock).
"""

import sys

if "/opt/trn_rl_repo" not in sys.path:
    sys.path.insert(0, "/opt/trn_rl_repo")

import numpy as np

B, R, Z, H = 2048, 512, 256, 512
ZR = Z + R
T = 16
NC = 8
BS = B // NC          # 256 batch per core
DT = 1.0 / T
SQDT = DT ** 0.5
P = 128
KZ = Z // P           # 2
KR = R // P           # 4
KH = H // P           # 4
MH = H // P           # 4
MZ = Z // P           # 2
NF = ZR // P          # 6

_CACHE = {}


def _build():
    import concourse.bacc as bacc
    import concourse.tile as tile
    from concourse import mybir

    F32 = mybir.dt.float32
    F32R = mybir.dt.float32r
    AF = mybir.ActivationFunctionType
    OP = mybir.AluOpType

    nc = bacc.Bacc("TRN2", target_bir_lowering=False, debug=False,
                   num_devices=NC)

    # ---- DRAM tensors (per-core views; weights replicated) ----
    d_wz = nc.dram_tensor("wz", [Z, H], F32R, kind="ExternalInput").ap()
    d_wr = nc.dram_tensor("wr", [R, H], F32R, kind="ExternalInput").ap()
    d_wh = nc.dram_tensor("wh", [H, H], F32R, kind="ExternalInput").ap()
    d_wo = nc.dram_tensor("wo", [H, Z], F32R, kind="ExternalInput").ap()
    d_wt = nc.dram_tensor("wt", [1, ZR], F32R, kind="ExternalInput").ap()
    d_ts = nc.dram_tensor("ts", [1, T], F32R, kind="ExternalInput").ap()
    d_bt = nc.dram_tensor("bt", [ZR, 1], F32, kind="ExternalInput").ap()
    d_b0 = nc.dram_tensor("b0", [H, 1], F32, kind="ExternalInput").ap()
    d_bh = nc.dram_tensor("bh", [H, 1], F32, kind="ExternalInput").ap()
    d_bo = nc.dram_tensor("bo", [Z, 1], F32, kind="ExternalInput").ap()
    d_rt = nc.dram_tensor("rT", [R, BS], F32R, kind="ExternalInput").ap()
    d_z0 = nc.dram_tensor("z0T", [Z, BS], F32R, kind="ExternalInput").ap()
    d_eps = nc.dram_tensor("epsT", [T, Z, BS], F32, kind="ExternalInput").ap()
    d_zs = nc.dram_tensor("zsT", [T, Z, BS], F32R, kind="ExternalOutput").ap()
    d_mus = nc.dram_tensor("musT", [T, Z, BS], F32, kind="ExternalOutput").ap()

    with tile.TileContext(nc) as tc:
        with tc.tile_pool(name="w", bufs=1) as wp, \
             tc.tile_pool(name="v", bufs=1) as vp, \
             tc.tile_pool(name="act", bufs=1) as ap_, \
             tc.tile_pool(name="st", bufs=2) as sp, \
             tc.tile_pool(name="ps", bufs=1, space="PSUM") as pp:

            # ---- loads, critical-path first ----
            ts = vp.tile([1, T], F32R, tag="ts", name="ts")
            nc.sync.dma_start(ts[:], d_ts[:])
            wt = vp.tile([1, ZR], F32R, tag="wt", name="wt")
            nc.sync.dma_start(wt[:], d_wt[:])
            bt = [vp.tile([P, 1], F32, tag=f"bt{f}", name=f"bt{f}")
                  for f in range(NF)]
            for f in range(NF):
                nc.sync.dma_start(bt[f][:], d_bt[f * P:(f + 1) * P, :])
            b0 = [vp.tile([P, 1], F32, tag=f"b0{m}", name=f"b0{m}")
                  for m in range(MH)]
            for m in range(MH):
                nc.sync.dma_start(b0[m][:], d_b0[m * P:(m + 1) * P, :])
            z = [sp.tile([P, BS], F32R, tag=f"z{k}", name=f"z{k}")
                 for k in range(KZ)]
            for k in range(KZ):
                nc.sync.dma_start(z[k][:], d_z0[k * P:(k + 1) * P, :])

            wz = [wp.tile([P, H], F32R, tag=f"wz{k}", name=f"wz{k}")
                  for k in range(KZ)]
            for k in range(KZ):
                nc.sync.dma_start(wz[k][:], d_wz[k * P:(k + 1) * P, :])
            wr = [wp.tile([P, H], F32R, tag=f"wr{k}", name=f"wr{k}")
                  for k in range(KR)]
            for k in range(KR):
                nc.sync.dma_start(wr[k][:], d_wr[k * P:(k + 1) * P, :])
            rt = [wp.tile([P, BS], F32R, tag=f"rt{k}", name=f"rt{k}")
                  for k in range(KR)]
            for k in range(KR):
                nc.sync.dma_start(rt[k][:], d_rt[k * P:(k + 1) * P, :])

            bh = [vp.tile([P, 1], F32, tag=f"bh{m}", name=f"bh{m}")
                  for m in range(MH)]
            for m in range(MH):
                nc.sync.dma_start(bh[m][:], d_bh[m * P:(m + 1) * P, :])
            bo = [vp.tile([P, 1], F32, tag=f"bo{m}", name=f"bo{m}")
                  for m in range(MZ)]
            for m in range(MZ):
                nc.sync.dma_start(bo[m][:], d_bo[m * P:(m + 1) * P, :])

            wh = [wp.tile([P, H], F32R, tag=f"wh{k}", name=f"wh{k}")
                  for k in range(KH)]
            for k in range(KH):
                nc.sync.dma_start(wh[k][:], d_wh[k * P:(k + 1) * P, :])
            wo = [wp.tile([P, Z], F32R, tag=f"wo{k}", name=f"wo{k}")
                  for k in range(KH)]
            for k in range(KH):
                nc.sync.dma_start(wo[k][:], d_wo[k * P:(k + 1) * P, :])

            # dt * bo as per-partition column (folded into y)
            dtbo = [vp.tile([P, 1], F32, tag=f"dtbo{m}", name=f"dtbo{m}")
                    for m in range(MZ)]
            for m in range(MZ):
                nc.scalar.activation(dtbo[m][:], bo[m][:], AF.Copy, scale=DT)

            # W0 row-tiles in feature order (z features then r features)
            w0 = wz + wr  # 6 tiles of [128, H]

            # ---- temb[f] = relu(Wt_f^T ts + bt_f) : [128, T] ----
            temb = [ap_.tile([P, T], F32R, tag=f"temb{f}", name=f"temb{f}")
                    for f in range(NF)]
            for f in range(NF):
                ps = pp.tile([P, T], F32, tag=f"pa{f % MH}", name=f"pt{f}")
                nc.tensor.matmul(ps[:], wt[0:1, f * P:(f + 1) * P], ts[:],
                                 start=True, stop=True)
                nc.scalar.activation(temb[f][:], ps[:], AF.Relu,
                                     bias=bt[f][:])

            # ---- c[m][:, t] = (temb_t @ W0 + b0)[m-tile] : [128, T] ----
            c = [ap_.tile([P, T], F32, tag=f"c{m}", name=f"c{m}")
                 for m in range(MH)]
            for m in range(MH):
                ps = pp.tile([P, T], F32, tag=f"pb{m}", name=f"pc{m}")
                for f in range(NF):
                    nc.tensor.matmul(ps[:], w0[f][:, m * P:(m + 1) * P],
                                     temb[f][:], start=(f == 0),
                                     stop=(f == NF - 1))
                nc.scalar.activation(c[m][:], ps[:], AF.Identity,
                                     bias=b0[m][:])

            # ---- rW[m] = (r @ W0[Z:]) tile, feature-major [128, BS] ----
            rwps = [pp.tile([P, BS], F32, tag=f"pa{m}", name=f"prw{m}")
                    for m in range(MH)]
            for k in range(KR):
                for m in range(MH):
                    nc.tensor.matmul(rwps[m][:],
                                     wr[k][:, m * P:(m + 1) * P], rt[k][:],
                                     start=(k == 0), stop=(k == KR - 1))
            rw = [ap_.tile([P, BS], F32, tag=f"rw{m}", name=f"rw{m}")
                  for m in range(MH)]
            for m in range(MH):
                nc.scalar.activation(rw[m][:], rwps[m][:], AF.Copy)

            # ---- the scan ----
            for t in range(T):
                eps = [sp.tile([P, BS], F32, tag=f"e{k}", name=f"e{k}",
                               bufs=4) for k in range(KZ)]
                for k in range(KZ):
                    nc.sync.dma_start(eps[k][:],
                                      d_eps[t, k * P:(k + 1) * P, :])

                # stage A: ps_a[m] = z @ Wz ; a = relu(ps_a + c_t + rW)
                ps_a = [pp.tile([P, BS], F32, tag=f"pa{m}",
                                name=f"pa{m}_{t}") for m in range(MH)]
                for k in range(KZ):
                    for m in range(MH):
                        nc.tensor.matmul(ps_a[m][:],
                                         wz[k][:, m * P:(m + 1) * P],
                                         z[k][:], start=(k == 0),
                                         stop=(k == KZ - 1))
                a = []
                for m in range(MH):
                    tmp = sp.tile([P, BS], F32, tag=f"tmp{m}",
                                  name=f"tmp{m}_{t}", bufs=1)
                    nc.vector.scalar_tensor_tensor(
                        tmp[:], ps_a[m][:], c[m][:, t:t + 1], rw[m][:],
                        op0=OP.add, op1=OP.add)
                    at = sp.tile([P, BS], F32R, tag=f"a{m}",
                                 name=f"a{m}_{t}", bufs=1)
                    nc.vector.tensor_scalar_max(at[:], tmp[:], 0.0)
                    a.append(at)

                # y = z + sqdt*eps + dt*bo  (off critical path)
                y = []
                for m in range(MZ):
                    y0 = sp.tile([P, BS], F32, tag=f"y0{m}",
                                 name=f"y0{m}_{t}", bufs=1)
                    nc.vector.scalar_tensor_tensor(
                        y0[:], eps[m][:], SQDT, z[m][:].bitcast(F32),
                        op0=OP.mult, op1=OP.add)
                    yt = sp.tile([P, BS], F32, tag=f"y{m}",
                                 name=f"y{m}_{t}", bufs=1)
                    nc.vector.tensor_scalar_add(yt[:], y0[:], dtbo[m][:])
                    y.append(yt)

                # stage B: g1 = a @ Wh + bh
                ps_b = [pp.tile([P, BS], F32, tag=f"pb{m}",
                                name=f"pb{m}_{t}") for m in range(MH)]
                for k in range(KH):
                    for m in range(MH):
                        nc.tensor.matmul(ps_b[m][:],
                                         wh[k][:, m * P:(m + 1) * P],
                                         a[k][:], start=(k == 0),
                                         stop=(k == KH - 1))
                g1 = []
                for m in range(MH):
                    g = sp.tile([P, BS], F32R, tag=f"g1{m}",
                                name=f"g1{m}_{t}", bufs=1)
                    nc.scalar.activation(g[:], ps_b[m][:], AF.Identity,
                                         bias=bh[m][:])
                    g1.append(g)

                # stage C: g2 = g1 @ Wh + bh
                ps_c = [pp.tile([P, BS], F32, tag=f"pa{m}",
                                name=f"pcc{m}_{t}") for m in range(MH)]
                for k in range(KH):
                    for m in range(MH):
                        nc.tensor.matmul(ps_c[m][:],
                                         wh[k][:, m * P:(m + 1) * P],
                                         g1[k][:], start=(k == 0),
                                         stop=(k == KH - 1))
                g2 = []
                for m in range(MH):
                    g = sp.tile([P, BS], F32R, tag=f"g2{m}",
                                name=f"g2{m}_{t}", bufs=1)
                    nc.scalar.activation(g[:], ps_c[m][:], AF.Identity,
                                         bias=bh[m][:])
                    g2.append(g)

                # stage D: s = g2 @ Wo ; z' = dt*s + y (one op on the
                # critical path), mu = z' - sqdt*eps (off path)
                ps_d = [pp.tile([P, BS], F32, tag=f"pb{m}",
                                name=f"pd{m}_{t}") for m in range(MZ)]
                for k in range(KH):
                    for m in range(MZ):
                        nc.tensor.matmul(ps_d[m][:],
                                         wo[k][:, m * P:(m + 1) * P],
                                         g2[k][:], start=(k == 0),
                                         stop=(k == KH - 1))
                z_new = []
                for m in range(MZ):
                    zn = sp.tile([P, BS], F32R, tag=f"z{m}",
                                 name=f"z{m}_{t + 1}")
                    nc.vector.scalar_tensor_tensor(
                        zn[:], ps_d[m][:], DT, y[m][:],
                        op0=OP.mult, op1=OP.add)
                    z_new.append(zn)
                for m in range(MZ):
                    mu = sp.tile([P, BS], F32, tag=f"mu{m}",
                                 name=f"mu{m}_{t}", bufs=2)
                    nc.vector.scalar_tensor_tensor(
                        mu[:], eps[m][:], -SQDT,
                        z_new[m][:].bitcast(F32),
                        op0=OP.mult, op1=OP.add)
                    nc.sync.dma_start(d_zs[t, m * P:(m + 1) * P, :],
                                      z_new[m][:])
                    nc.sync.dma_start(d_mus[t, m * P:(m + 1) * P, :],
                                      mu[:])
                z = z_new

    nc.compile()
    return nc


def _get_nc():
    if "nc" not in _CACHE:
        _CACHE["nc"] = _build()
    return _CACHE["nc"]


def _in_maps(inputs):
    f32 = lambda x: np.ascontiguousarray(np.asarray(x, dtype=np.float32))
    r = f32(inputs["r"])
    noise0 = f32(inputs["noise0"])
    noise = f32(inputs["noise"])
    W0 = f32(inputs["W0"])
    b0 = f32(inputs["b0"])
    Wh = f32(inputs["Wh"])
    bh = f32(inputs["bh"])
    Wo = f32(inputs["Wo"])
    bo = f32(inputs["bo"])
    Wt = f32(inputs["Wt"])
    bt = f32(inputs["bt"])

    shared = {
        "wz": W0[:Z], "wr": W0[Z:], "wh": Wh, "wo": Wo,
        "wt": Wt.reshape(1, ZR),
        "ts": (np.arange(1, T + 1, dtype=np.float32)
               * np.float32(DT)).reshape(1, T),
        "bt": bt.reshape(ZR, 1),
        "b0": b0.reshape(H, 1),
        "bh": bh.reshape(H, 1),
        "bo": bo.reshape(Z, 1),
    }
    rT = np.ascontiguousarray(r.T)                         # [R, B]
    z0T = np.ascontiguousarray(noise0.T)                   # [Z, B]
    epsT = np.ascontiguousarray(noise.transpose(0, 2, 1))  # [T, Z, B]
    maps = []
    for cix in range(NC):
        s = slice(cix * BS, (cix + 1) * BS)
        m = dict(shared)
        m["rT"] = np.ascontiguousarray(rT[:, s])
        m["z0T"] = np.ascontiguousarray(z0T[:, s])
        m["epsT"] = np.ascontiguousarray(epsT[:, :, s])
        maps.append(m)
    return maps, noise0


def _run(inputs, **run_kwargs):
    from concourse.bass_utils import run_bass_kernel_spmd
    nc = _get_nc()
    maps, noise0 = _in_maps(inputs)
    res = run_bass_kernel_spmd(nc, maps, core_ids=list(range(NC)),
                               **run_kwargs)
    out = np.empty((3, T + 1, B, Z), np.float32)
    out[0, 0] = noise0
    out[1, 0] = 0.0
    out[2, 0] = 1.0
    out[2, 1:] = np.float32(SQDT)
    for cix in range(NC):
        s = slice(cix * BS, (cix + 1) * BS)
        out[0, 1:, s, :] = res.results[cix]["zsT"].transpose(0, 2, 1)
        out[1, 1:, s, :] = res.results[cix]["musT"].transpose(0, 2, 1)
    return out, res


def kernel(**inputs) -> np.ndarray:
    out, _ = _run(inputs)
    return out
